# revision 11
# baseline (speedup 1.0000x reference)
"""Trainium2 Bass kernel for the GRU seq2seq AR model.

Model (reference): GRU encoder over S=1024 steps, then T=256 autoregressive
decoder steps (teacher_forcing_rate=0, so decoder input is always its own
previous output y = Wl @ h + bl).

Strategy:
  - Pure data parallel: batch 512 sharded 8 ways (64 rows/core), weights
    replicated, zero collectives.
  - Per step, fused matmuls compute gate pre-activations into four psum
    tiles gr|gz|gn|gh (each [128, 4x64batch]), K = [x(64); 1; h(512)]
    split in 5 k-tiles.  A ones-row folds all biases in.
  - Gate math uses the sigmoid_and_others ACT table (sigmoid+tanh, one
    table set):  r = sig(gr), nz = sig(-gz), u2 = r*gh, an = u2+gn (psum),
    tn = tanh(an), s = h-tn, p = nz*s, h' = h-p.  Split in H-halves for
    latency pipelining with the PE stream.
  - Decoder: Wl folded into gate weights (inp = Wl@h + bl always), so no
    serial y-feedback; y is output-only.
  - bf16 matmul inputs, fp32 PSUM, bf16 SBUF-side gate tensors.

Layouts (per core, BC = 64):
  w    DRAM [10, 128, 2112] bf16: k-tiles 0:5 encoder, 5:10 decoder.
  xh   DRAM [65, S*BC] bf16: rows 0:64 = x[t].T steps, row 64 = ones.
  y    DRAM [64, (T+1)*BC] f32: slot d holds Wl@h^{(d)}+bl ([I, BC] each).
"""

import numpy as np
import ml_dtypes

B, S, I, H, T = 512, 1024, 64, 512, 256
NCORES = 8
BC = B // NCORES
BF16 = ml_dtypes.bfloat16

# M-column blocks inside each 2112-col weight tile
_R0, _Z0, _N0, _H0, _Y0 = 0, 512, 1024, 1536, 2048
WCOLS = 2112


def _build_weights(Wi, Wh, bi, bh, Wl, bl):
    """lhsT tiles [10, 128, 2112] fp32 -> bf16.
    K rows: k0 = [x(64); ones(1)], k1..k4 = h chunks of 128.
    Tiles 0:5 = encoder, 5:10 = decoder (Wl folded).  No gate pre-scaling:
    r/z use real sigmoid."""
    w = np.zeros((10, 128, WCOLS), np.float32)

    def fill(base, Wx, bx_r, bx_z, bx_n, Whh, Win_, x_has_w):
        if x_has_w:
            w[base, 0:64, _R0:_R0 + 512] = Wx.T[:, 0:512]
            w[base, 0:64, _Z0:_Z0 + 512] = Wx.T[:, 512:1024]
            w[base, 0:64, _N0:_N0 + 512] = Wx.T[:, 1024:1536]
        w[base, 64, _R0:_R0 + 512] = bx_r
        w[base, 64, _Z0:_Z0 + 512] = bx_z
        w[base, 64, _N0:_N0 + 512] = bx_n
        w[base, 64, _H0:_H0 + 512] = bh[1024:1536]
        w[base, 64, _Y0:_Y0 + 64] = bl
        for c in range(4):
            hs = slice(128 * c, 128 * (c + 1))
            w[base + 1 + c, :, _R0:_R0 + 512] = Whh.T[hs, 0:512]
            w[base + 1 + c, :, _Z0:_Z0 + 512] = Whh.T[hs, 512:1024]
            if Win_ is not None:
                w[base + 1 + c, :, _N0:_N0 + 512] = Win_.T[hs, :]
            w[base + 1 + c, :, _H0:_H0 + 512] = Wh[1024:1536].T[hs, :]
            w[base + 1 + c, :, _Y0:_Y0 + 64] = Wl.T[hs, :]

    # encoder: gi from x via k0; gh from h
    fill(0, Wi, (bi + bh)[0:512], (bi + bh)[512:1024], bi[1024:1536],
         Wh[0:1024], None, x_has_w=True)
    # decoder: inp = Wl@h + bl folded -> all through h rows
    Wc = Wh[0:1024] + Wi[0:1024] @ Wl
    Win = Wi[1024:1536] @ Wl
    fill(5, Wi,
         (bi + bh)[0:512] + Wi[0:512] @ bl,
         (bi + bh)[512:1024] + Wi[512:1024] @ bl,
         bi[1024:1536] + Wi[1024:1536] @ bl,
         Wc, Win, x_has_w=False)
    return w.astype(BF16)


def _build_x(xc):
    """xc [BC, S, I] -> [65, S*BC] bf16 with ones row."""
    s = xc.shape[1]
    xt = np.ones((65, s, BC), np.float32)
    xt[0:64] = xc.transpose(2, 1, 0)
    return np.ascontiguousarray(xt.reshape(65, s * BC)).astype(BF16)


def build_program(s_steps=S, t_steps=T, ue=32, ud=32, use_loops=True):
    """Build the Bass/Tile program (shared by all 8 cores)."""
    from contextlib import ExitStack
    import concourse.bass as bass
    import concourse.bacc as bacc
    import concourse.mybir as mybir
    import concourse.tile as tile

    f32 = mybir.dt.float32
    bf16 = mybir.dt.bfloat16
    TANH = mybir.ActivationFunctionType.Tanh
    SIG = mybir.ActivationFunctionType.Sigmoid
    MUL = mybir.AluOpType.mult
    ADD = mybir.AluOpType.add
    SUB = mybir.AluOpType.subtract

    assert s_steps % ue == 0 and t_steps % ud == 0

    nc = bacc.Bacc("TRN2", target_bir_lowering=False, debug=False,
                   num_devices=NCORES)
    w_ext = nc.declare_dram_parameter("w", [10, 128, WCOLS], bf16, isOutput=False)
    x_ext = nc.declare_dram_parameter("xh", [65, s_steps * BC], bf16, isOutput=False)
    y_ext = nc.declare_dram_parameter("y", [64, (t_steps + 1) * BC], f32, isOutput=True)

    with ExitStack() as ctx:
        tc = ctx.enter_context(tile.TileContext(nc))
        state = ctx.enter_context(tc.tile_pool(name="state", bufs=1))
        wpool = ctx.enter_context(tc.tile_pool(name="wpool", bufs=1))
        xpool = ctx.enter_context(tc.tile_pool(name="xpool", bufs=2))
        ypool = ctx.enter_context(tc.tile_pool(name="ypool", bufs=2))
        gp = ctx.enter_context(tc.tile_pool(name="gates", bufs=2))
        psum = ctx.enter_context(tc.tile_pool(name="psum", bufs=2, space="PSUM"))

        wte, wtd = [], []
        for k in range(10):
            t_ = wpool.tile([128, WCOLS], bf16, tag=f"w{k}")
            nc.sync.dma_start(t_[:], w_ext[k, :, :])
            (wte if k < 5 else wtd).append(t_)

        hbf = state.tile([128, 256], bf16, tag="hbf")    # h.T chunks (bf16)
        rhs0d = state.tile([65, BC], bf16, tag="rhs0d")  # decoder k0 = [0...; 1]
        nc.vector.memset(hbf[:], 0.0)
        nc.vector.memset(rhs0d[:], 0.0)
        nc.vector.memset(rhs0d[64:65, :], 1.0)

        HA, HB = slice(0, 128), slice(128, 256)

        def emit_mms(grz, gnh, gy, rhs0, wt, enc, want_y):
            """Gate matmuls.  Each psum col-chunk [*, 64m:64m+64] is one
            accumulation group: start on its k0 MM, stop on its k4 MM.
            k1/k2 groups (read hbf half-A) emitted before k3/k4 groups
            (half-B) so the PE can start while half-B is still computing;
            r before h_n before z so the serial chain head finishes first.
            psum col layout: grz = [r(256) | z(256)], gnh = [n(256) | h(256)]."""
            hk = lambda k: hbf[:, (k - 1) * 64:k * 64]
            k0 = wt[0][0:65, :]

            def mm4(ps, pcol0, wcol0, k, start, stop):
                # stop applies only to the last (m=3) MM of the group
                if k == 0:
                    for m in range(4):
                        nc.tensor.matmul(ps[:, pcol0 + 64 * m:pcol0 + 64 * m + 64],
                                         k0[:, wcol0 + 128 * m:wcol0 + 128 * m + 128],
                                         rhs0, start=start, stop=(stop and m == 3))
                else:
                    for m in range(4):
                        nc.tensor.matmul(ps[:, pcol0 + 64 * m:pcol0 + 64 * m + 64],
                                         wt[k][:, wcol0 + 128 * m:wcol0 + 128 * m + 128],
                                         hk(k), start=start, stop=(stop and m == 3))

            # phase A: k0 (x rows for encoder; bias row always).
            # start=True ONLY on the first MM per psum bank (it clears the
            # whole bank); stop=True only on the last MM per bank.
            def mm4_first(ps, pcol0, wcol0):
                nc.tensor.matmul(ps[:, pcol0:pcol0 + 64],
                                 k0[:, wcol0:wcol0 + 128],
                                 rhs0, start=True, stop=False)
                for m in range(1, 4):
                    nc.tensor.matmul(ps[:, pcol0 + 64 * m:pcol0 + 64 * m + 64],
                                     k0[:, wcol0 + 128 * m:wcol0 + 128 * m + 128],
                                     rhs0, start=False, stop=False)

            mm4_first(grz, 0, _R0)
            mm4(grz, 256, _Z0, 0, False, False)
            mm4_first(gnh, 0, _N0)
            mm4(gnh, 256, _H0, 0, False, False)
            if want_y:
                nc.tensor.matmul(gy[:, :], k0[:, _Y0:_Y0 + 64],
                                 rhs0, start=True, stop=False)
            # k1/k2 groups (hbf half-A readers)
            for k in (1, 2):
                mm4(grz, 0, _R0, k, False, False)
            for k in (1, 2):
                mm4(gnh, 256, _H0, k, False, False)
            for k in (1, 2):
                mm4(grz, 256, _Z0, k, False, False)
            if not enc:
                for k in (1, 2):
                    mm4(gnh, 0, _N0, k, False, False)
            if want_y:
                for k in (1, 2):
                    nc.tensor.matmul(gy[:, :], wt[k][:, _Y0:_Y0 + 64],
                                     hk(k), start=False, stop=False)
            # k3/k4 groups (hbf half-B readers), chain-critical first.
            # stop=True only on the last MM per psum bank.
            mm4(grz, 0, _R0, 3, False, False)
            mm4(grz, 0, _R0, 4, False, False)
            mm4(gnh, 256, _H0, 3, False, False)
            mm4(gnh, 256, _H0, 4, False, enc)
            mm4(grz, 256, _Z0, 3, False, False)
            mm4(grz, 256, _Z0, 4, False, True)
            if not enc:
                mm4(gnh, 0, _N0, 3, False, False)
                mm4(gnh, 0, _N0, 4, False, True)
            if want_y:
                for k in (3, 4):
                    nc.tensor.matmul(gy[:, :], wt[k][:, _Y0:_Y0 + 64],
                                     hk(k), start=False, stop=(k == 4))

        def emit_gates(grz, gnh, an, gy, ytile=None, yslot=0):
            gr = grz[:, 0:256]
            gz = grz[:, 256:512]
            gn = gnh[:, 0:256]
            gh = gnh[:, 256:512]
            """Gate math in H-halves:
              r = sig(gr); nz = sig(-gz); u2 = r*gh; an = u2+gn (psum);
              tn = tanh(an); s = h-tn; p = nz*s; h' = h-p."""
            r = gp.tile([128, 256], bf16, tag="r")
            nz = gp.tile([128, 256], bf16, tag="nz")
            u2 = gp.tile([128, 256], bf16, tag="u2")
            tn = gp.tile([128, 256], bf16, tag="tn")
            s = gp.tile([128, 256], bf16, tag="s")
            p = gp.tile([128, 256], bf16, tag="p")

            # ACT: rA, rB early; nz and tanh interleaved per half
            nc.scalar.activation(r[:, HA], gr[:, HA], SIG)
            nc.scalar.activation(r[:, HB], gr[:, HB], SIG)
            # DVE head: u2/an per half
            nc.vector.tensor_tensor(u2[:, HA], r[:, HA], gh[:, HA], MUL)
            nc.vector.tensor_tensor(an[:, HA], u2[:, HA], gn[:, HA], ADD)
            nc.vector.tensor_tensor(u2[:, HB], r[:, HB], gh[:, HB], MUL)
            nc.vector.tensor_tensor(an[:, HB], u2[:, HB], gn[:, HB], ADD)
            # ACT tail
            nc.scalar.activation(nz[:, HA], gz[:, HA], SIG, scale=-1.0)
            nc.scalar.activation(tn[:, HA], an[:, HA], TANH)
            nc.scalar.activation(nz[:, HB], gz[:, HB], SIG, scale=-1.0)
            nc.scalar.activation(tn[:, HB], an[:, HB], TANH)
            # DVE tail: h' = h - nz*(h - tn)
            nc.vector.tensor_tensor(s[:, HA], hbf[:, HA], tn[:, HA], SUB)
            nc.vector.tensor_tensor(p[:, HA], nz[:, HA], s[:, HA], MUL)
            nc.vector.tensor_tensor(hbf[:, HA], hbf[:, HA], p[:, HA], SUB)
            nc.vector.tensor_tensor(s[:, HB], hbf[:, HB], tn[:, HB], SUB)
            nc.vector.tensor_tensor(p[:, HB], nz[:, HB], s[:, HB], MUL)
            nc.vector.tensor_tensor(hbf[:, HB], hbf[:, HB], p[:, HB], SUB)
            if ytile is not None:
                nc.vector.tensor_copy(
                    ytile[:, yslot * BC:(yslot + 1) * BC], gy[:, :])

        def enc_step(rhs0):
            grz = psum.tile([128, 512], f32, tag="grz")
            gnh = psum.tile([128, 512], f32, tag="gnh")
            an = psum.tile([128, 256], f32, tag="an")
            emit_mms(grz, gnh, None, rhs0, wte, enc=True, want_y=False)
            emit_gates(grz, gnh, an, None)

        def dec_step(ytile, yslot):
            grz = psum.tile([128, 512], f32, tag="grz")
            gnh = psum.tile([128, 512], f32, tag="gnh")
            an = psum.tile([128, 256], f32, tag="an")
            gy = psum.tile([64, 64], f32, tag="gy")
            emit_mms(grz, gnh, gy, rhs0d[0:65, :], wtd, enc=False, want_y=True)
            emit_gates(grz, gnh, an, gy, ytile=ytile, yslot=yslot)

        PE = mybir.EngineType.PE
        DVE = mybir.EngineType.DVE

        # ---- encoder ----
        if use_loops:
            with tc.For_i(0, s_steps * BC, ue * BC, hint_engines=(PE, DVE)) as iv:
                xch = xpool.tile([65, ue * BC], bf16, tag="xch")
                nc.sync.dma_start(xch[:], x_ext[:, bass.ds(iv, ue * BC)])
                for j in range(ue):
                    enc_step(xch[:, j * BC:(j + 1) * BC])
        else:
            for i0 in range(0, s_steps, ue):
                xch = xpool.tile([65, ue * BC], bf16, tag="xch")
                nc.sync.dma_start(xch[:], x_ext[:, i0 * BC:(i0 + ue) * BC])
                for j in range(ue):
                    enc_step(xch[:, j * BC:(j + 1) * BC])

        # ---- decoder (no bridge needed: Wl folded, no y feedback) ----
        if use_loops:
            with tc.For_i(0, t_steps * BC, ud * BC, hint_engines=(PE, DVE)) as iv:
                yt = ypool.tile([64, ud * BC], f32, tag="yt")
                for j in range(ud):
                    dec_step(yt, j)
                nc.sync.dma_start(y_ext[:, bass.ds(iv, ud * BC)], yt[:])
        else:
            for d0 in range(0, t_steps, ud):
                yt = ypool.tile([64, ud * BC], f32, tag="yt")
                for j in range(ud):
                    dec_step(yt, j)
                nc.sync.dma_start(y_ext[:, d0 * BC:(d0 + ud) * BC], yt[:])

        # ---- tail: y for the final hidden state -> slot T ----
        gy_t = psum.tile([64, 64], f32, tag="gy")
        nc.tensor.matmul(gy_t[:, :], wtd[0][0:65, _Y0:_Y0 + 64],
                         rhs0d[0:65, :], start=True, stop=False)
        for k in range(1, 5):
            nc.tensor.matmul(gy_t[:, :], wtd[k][:, _Y0:_Y0 + 64],
                             hbf[:, (k - 1) * 64:k * 64], start=False, stop=(k == 4))
        ylast = ypool.tile([64, BC], f32, tag="ylast")
        nc.vector.tensor_copy(ylast[:], gy_t[:, :])
        nc.sync.dma_start(y_ext[:, t_steps * BC:(t_steps + 1) * BC], ylast[:])

    nc.compile()
    return nc


def run(nc, w_bf, x_cores, trace=False):
    """Execute on 8 cores; returns per-core y arrays and BassKernelResults."""
    from concourse.bass_utils import run_bass_kernel_spmd
    in_maps = [{"w": w_bf, "xh": x_cores[c]} for c in range(NCORES)]
    res = run_bass_kernel_spmd(nc, in_maps, core_ids=list(range(NCORES)),
                               trace=trace)
    return [res.results[c]["y"] for c in range(NCORES)], res


_NC_CACHE = {}


def kernel(x, Wi, Wh, bi, bh, Wl, bl, targets=None, target_seq_len=T,
           teacher_forcing_rate=0, **_unused):
    x = np.asarray(x, np.float32)
    assert x.shape == (B, S, I), x.shape
    assert int(target_seq_len) == T
    w_bf = _build_weights(np.asarray(Wi, np.float32), np.asarray(Wh, np.float32),
                          np.asarray(bi, np.float32), np.asarray(bh, np.float32),
                          np.asarray(Wl, np.float32), np.asarray(bl, np.float32))
    x_cores = [_build_x(x[c * BC:(c + 1) * BC]) for c in range(NCORES)]

    key = (S, T)
    if key not in _NC_CACHE:
        _NC_CACHE[key] = build_program(S, T)
    ys, _ = run(_NC_CACHE[key], w_bf, x_cores)

    out = np.empty((B, T, I), np.float32)
    for c in range(NCORES):
        yc = ys[c].reshape(64, T + 1, BC)[:, 1:, :]   # [I, T, BC]
        out[c * BC:(c + 1) * BC] = yc.transpose(2, 1, 0)
    return out


if __name__ == "__main__":
    import reference
    inputs = reference.setup_inputs()
    out = kernel(**{k: np.asarray(v) if hasattr(v, "shape") else v
                    for k, v in inputs.items()})
    print("kernel out", out.shape, out.dtype)


# revision 15
# speedup vs baseline: 1.1357x; 1.1357x over previous
"""Trainium2 Bass kernel for the GRU seq2seq AR model.

Model (reference): GRU encoder over S=1024 steps, then T=256 autoregressive
decoder steps (teacher_forcing_rate=0, so decoder input is always its own
previous output y = Wl @ h + bl).

Strategy:
  - Pure data parallel: batch 512 sharded 8 ways (64 rows/core), weights
    replicated, zero collectives.
  - Per step, fused matmuls compute gate pre-activations into four psum
    tiles gr|gz|gn|gh (each [128, 4x64batch]), K = [x(64); 1; h(512)]
    split in 5 k-tiles.  A ones-row folds all biases in.
  - Gate math uses the sigmoid_and_others ACT table (sigmoid+tanh, one
    table set):  r = sig(gr), nz = sig(-gz), u2 = r*gh, an = u2+gn (psum),
    tn = tanh(an), s = h-tn, p = nz*s, h' = h-p.  Split in H-halves for
    latency pipelining with the PE stream.
  - Decoder: Wl folded into gate weights (inp = Wl@h + bl always), so no
    serial y-feedback; y is output-only.
  - bf16 matmul inputs, fp32 PSUM, bf16 SBUF-side gate tensors.

Layouts (per core, BC = 64):
  w    DRAM [10, 128, 2112] bf16: k-tiles 0:5 encoder, 5:10 decoder.
  xh   DRAM [65, S*BC] bf16: rows 0:64 = x[t].T steps, row 64 = ones.
  y    DRAM [64, (T+1)*BC] f32: slot d holds Wl@h^{(d)}+bl ([I, BC] each).
"""

import numpy as np
import ml_dtypes

B, S, I, H, T = 512, 1024, 64, 512, 256
NCORES = 8
BC = B // NCORES
BF16 = ml_dtypes.bfloat16

# M-column blocks inside each 2112-col weight tile
_R0, _Z0, _N0, _H0, _Y0 = 0, 512, 1024, 1536, 2048
WCOLS = 2112


def _build_weights(Wi, Wh, bi, bh, Wl, bl):
    """lhsT tiles [10, 128, 2112] fp32 -> bf16.
    K rows: k0 = [x(64); ones(1)], k1..k4 = h chunks of 128.
    Tiles 0:5 = encoder, 5:10 = decoder (Wl folded).  No gate pre-scaling:
    r/z use real sigmoid."""
    w = np.zeros((10, 128, WCOLS), np.float32)

    def fill(base, Wx, bx_r, bx_z, bx_n, Whh, Win_, x_has_w):
        if x_has_w:
            w[base, 0:64, _R0:_R0 + 512] = Wx.T[:, 0:512]
            w[base, 0:64, _Z0:_Z0 + 512] = Wx.T[:, 512:1024]
            w[base, 0:64, _N0:_N0 + 512] = Wx.T[:, 1024:1536]
        w[base, 64, _R0:_R0 + 512] = bx_r
        w[base, 64, _Z0:_Z0 + 512] = bx_z
        w[base, 64, _N0:_N0 + 512] = bx_n
        w[base, 64, _H0:_H0 + 512] = bh[1024:1536]
        w[base, 64, _Y0:_Y0 + 64] = bl
        for c in range(4):
            hs = slice(128 * c, 128 * (c + 1))
            w[base + 1 + c, :, _R0:_R0 + 512] = Whh.T[hs, 0:512]
            w[base + 1 + c, :, _Z0:_Z0 + 512] = Whh.T[hs, 512:1024]
            if Win_ is not None:
                w[base + 1 + c, :, _N0:_N0 + 512] = Win_.T[hs, :]
            w[base + 1 + c, :, _H0:_H0 + 512] = Wh[1024:1536].T[hs, :]
            w[base + 1 + c, :, _Y0:_Y0 + 64] = Wl.T[hs, :]

    # encoder: gi from x via k0; gh from h
    fill(0, Wi, (bi + bh)[0:512], (bi + bh)[512:1024], bi[1024:1536],
         Wh[0:1024], None, x_has_w=True)
    # decoder: inp = Wl@h + bl folded -> all through h rows
    Wc = Wh[0:1024] + Wi[0:1024] @ Wl
    Win = Wi[1024:1536] @ Wl
    fill(5, Wi,
         (bi + bh)[0:512] + Wi[0:512] @ bl,
         (bi + bh)[512:1024] + Wi[512:1024] @ bl,
         bi[1024:1536] + Wi[1024:1536] @ bl,
         Wc, Win, x_has_w=False)
    return w.astype(BF16)


def _build_x(xc):
    """xc [BC, S, I] -> [65, S*BC] bf16 with ones row."""
    s = xc.shape[1]
    xt = np.ones((65, s, BC), np.float32)
    xt[0:64] = xc.transpose(2, 1, 0)
    return np.ascontiguousarray(xt.reshape(65, s * BC)).astype(BF16)


def build_program(s_steps=S, t_steps=T, ue=32, ud=32, use_loops=True):
    """Build the Bass/Tile program (shared by all 8 cores)."""
    from contextlib import ExitStack
    import concourse.bass as bass
    import concourse.bacc as bacc
    import concourse.mybir as mybir
    import concourse.tile as tile

    f32 = mybir.dt.float32
    bf16 = mybir.dt.bfloat16
    TANH = mybir.ActivationFunctionType.Tanh
    SIG = mybir.ActivationFunctionType.Sigmoid
    MUL = mybir.AluOpType.mult
    ADD = mybir.AluOpType.add
    SUB = mybir.AluOpType.subtract

    assert s_steps % ue == 0 and t_steps % ud == 0

    nc = bacc.Bacc("TRN2", target_bir_lowering=False, debug=False,
                   num_devices=NCORES)
    w_ext = nc.declare_dram_parameter("w", [10, 128, WCOLS], bf16, isOutput=False)
    x_ext = nc.declare_dram_parameter("xh", [65, s_steps * BC], bf16, isOutput=False)
    y_ext = nc.declare_dram_parameter("y", [64, (t_steps + 1) * BC], f32, isOutput=True)

    with ExitStack() as ctx:
        tc = ctx.enter_context(tile.TileContext(nc))
        state = ctx.enter_context(tc.tile_pool(name="state", bufs=1))
        wpool = ctx.enter_context(tc.tile_pool(name="wpool", bufs=1))
        xpool = ctx.enter_context(tc.tile_pool(name="xpool", bufs=2))
        ypool = ctx.enter_context(tc.tile_pool(name="ypool", bufs=2))
        gp = ctx.enter_context(tc.tile_pool(name="gates", bufs=2))
        psum = ctx.enter_context(tc.tile_pool(name="psum", bufs=2, space="PSUM"))

        wte, wtd = [], []
        for k in range(10):
            t_ = wpool.tile([128, WCOLS], bf16, tag=f"w{k}")
            nc.sync.dma_start(t_[:], w_ext[k, :, :])
            (wte if k < 5 else wtd).append(t_)

        hbf = state.tile([128, 256], bf16, tag="hbf")    # h.T chunks (bf16)
        rhs0d = state.tile([65, BC], bf16, tag="rhs0d")  # decoder k0 = [0...; 1]
        nc.vector.memset(hbf[:], 0.0)
        nc.vector.memset(rhs0d[:], 0.0)
        nc.vector.memset(rhs0d[64:65, :], 1.0)

        HA, HB = slice(0, 128), slice(128, 256)

        def emit_mms(b_r, b_hn, b_z, b_ay, rhs0, wt, enc, want_y):
            """Gate matmuls.  Bank assignment is by chain deadline: b_r
            holds only r (its accumulation group closes after 32 MMs so the
            sigmoid can start), b_hn holds h_n|n, b_z holds z, b_ay holds
            gy (+the DVE-written an region).  start=True only on the first
            MM per bank (it clears the whole bank); stop=True only on the
            last MM per bank (readers gate on the group end)."""
            hk = lambda k: hbf[:, (k - 1) * 64:k * 64]
            k0 = wt[0][0:65, :]

            def mm4(ps, pcol0, wcol0, k, start, stop):
                # start applies only to the first (m=0) MM, stop to the last
                for m in range(4):
                    lhs = (k0 if k == 0 else wt[k])
                    rr = (rhs0 if k == 0 else hk(k))
                    nc.tensor.matmul(ps[:, pcol0 + 64 * m:pcol0 + 64 * m + 64],
                                     lhs[:, wcol0 + 128 * m:wcol0 + 128 * m + 128],
                                     rr, start=(start and m == 0),
                                     stop=(stop and m == 3))

            # k0 (x rows for encoder; bias row always); first MM per bank
            mm4(b_r, 0, _R0, 0, True, False)
            mm4(b_hn, 0, _H0, 0, True, False)
            mm4(b_z, 0, _Z0, 0, True, False)
            mm4(b_hn, 256, _N0, 0, False, False)
            if want_y:
                nc.tensor.matmul(b_ay[0:64, 256:320], k0[:, _Y0:_Y0 + 64],
                                 rhs0, start=True, stop=False)
            # k1/k2 groups (hbf half-A readers)
            for k in (1, 2):
                mm4(b_r, 0, _R0, k, False, False)
            for k in (1, 2):
                mm4(b_hn, 0, _H0, k, False, False)
            for k in (1, 2):
                mm4(b_z, 0, _Z0, k, False, False)
            if not enc:
                for k in (1, 2):
                    mm4(b_hn, 256, _N0, k, False, False)
            if want_y:
                for k in (1, 2):
                    nc.tensor.matmul(b_ay[0:64, 256:320], wt[k][:, _Y0:_Y0 + 64],
                                     hk(k), start=False, stop=False)
            # k3/k4 groups (hbf half-B readers): r first (chain head), then
            # h_n (u2 input), then z / n / y which have later deadlines.
            mm4(b_r, 0, _R0, 3, False, False)
            mm4(b_r, 0, _R0, 4, False, True)
            mm4(b_hn, 0, _H0, 3, False, False)
            mm4(b_hn, 0, _H0, 4, False, enc)
            mm4(b_z, 0, _Z0, 3, False, False)
            mm4(b_z, 0, _Z0, 4, False, True)
            if not enc:
                mm4(b_hn, 256, _N0, 3, False, False)
                mm4(b_hn, 256, _N0, 4, False, True)
            if want_y:
                for k in (3, 4):
                    nc.tensor.matmul(b_ay[0:64, 256:320], wt[k][:, _Y0:_Y0 + 64],
                                     hk(k), start=False, stop=(k == 4))

        def emit_gates(b_r, b_hn, b_z, b_ay, ytile=None, yslot=0):
            gr = b_r[:, 0:256]
            gh = b_hn[:, 0:256]
            gn = b_hn[:, 256:512]
            gz = b_z[:, 0:256]
            an = b_ay[:, 0:256]
            gy = b_ay[0:64, 256:320]
            """Gate math in H-halves:
              r = sig(gr); nz = sig(-gz); u2 = r*gh; an = u2+gn (psum);
              tn = tanh(an); s = h-tn; p = nz*s; h' = h-p."""
            r = gp.tile([128, 256], bf16, tag="r")
            nz = gp.tile([128, 256], bf16, tag="nz")
            u2 = gp.tile([128, 256], bf16, tag="u2")
            tn = gp.tile([128, 256], bf16, tag="tn")
            s = gp.tile([128, 256], bf16, tag="s")
            p = gp.tile([128, 256], bf16, tag="p")

            # ACT: rA, rB early; nz and tanh interleaved per half
            nc.scalar.activation(r[:, HA], gr[:, HA], SIG)
            nc.scalar.activation(r[:, HB], gr[:, HB], SIG)
            # DVE head: u2/an per half
            nc.vector.tensor_tensor(u2[:, HA], r[:, HA], gh[:, HA], MUL)
            nc.vector.tensor_tensor(an[:, HA], u2[:, HA], gn[:, HA], ADD)
            nc.vector.tensor_tensor(u2[:, HB], r[:, HB], gh[:, HB], MUL)
            nc.vector.tensor_tensor(an[:, HB], u2[:, HB], gn[:, HB], ADD)
            # ACT tail
            nc.scalar.activation(nz[:, HA], gz[:, HA], SIG, scale=-1.0)
            nc.scalar.activation(tn[:, HA], an[:, HA], TANH)
            nc.scalar.activation(nz[:, HB], gz[:, HB], SIG, scale=-1.0)
            nc.scalar.activation(tn[:, HB], an[:, HB], TANH)
            # DVE tail: h' = h - nz*(h - tn)
            nc.vector.tensor_tensor(s[:, HA], hbf[:, HA], tn[:, HA], SUB)
            nc.vector.tensor_tensor(p[:, HA], nz[:, HA], s[:, HA], MUL)
            nc.vector.tensor_tensor(hbf[:, HA], hbf[:, HA], p[:, HA], SUB)
            nc.vector.tensor_tensor(s[:, HB], hbf[:, HB], tn[:, HB], SUB)
            nc.vector.tensor_tensor(p[:, HB], nz[:, HB], s[:, HB], MUL)
            nc.vector.tensor_tensor(hbf[:, HB], hbf[:, HB], p[:, HB], SUB)
            if ytile is not None:
                nc.vector.tensor_copy(
                    ytile[:, yslot * BC:(yslot + 1) * BC], gy[:, :])

        def alloc_psum():
            b_r = psum.tile([128, 256], f32, tag="b_r")
            b_hn = psum.tile([128, 512], f32, tag="b_hn")
            b_z = psum.tile([128, 256], f32, tag="b_z")
            b_ay = psum.tile([128, 512], f32, tag="b_ay")
            return b_r, b_hn, b_z, b_ay

        def enc_step(rhs0):
            b_r, b_hn, b_z, b_ay = alloc_psum()
            emit_mms(b_r, b_hn, b_z, b_ay, rhs0, wte, enc=True, want_y=False)
            emit_gates(b_r, b_hn, b_z, b_ay)

        def dec_step(ytile, yslot):
            b_r, b_hn, b_z, b_ay = alloc_psum()
            emit_mms(b_r, b_hn, b_z, b_ay, rhs0d[0:65, :], wtd, enc=False, want_y=True)
            emit_gates(b_r, b_hn, b_z, b_ay, ytile=ytile, yslot=yslot)

        PE = mybir.EngineType.PE
        DVE = mybir.EngineType.DVE

        # ---- encoder ----
        if use_loops:
            with tc.For_i(0, s_steps * BC, ue * BC, hint_engines=(PE, DVE)) as iv:
                xch = xpool.tile([65, ue * BC], bf16, tag="xch")
                nc.sync.dma_start(xch[:], x_ext[:, bass.ds(iv, ue * BC)])
                for j in range(ue):
                    enc_step(xch[:, j * BC:(j + 1) * BC])
        else:
            for i0 in range(0, s_steps, ue):
                xch = xpool.tile([65, ue * BC], bf16, tag="xch")
                nc.sync.dma_start(xch[:], x_ext[:, i0 * BC:(i0 + ue) * BC])
                for j in range(ue):
                    enc_step(xch[:, j * BC:(j + 1) * BC])

        # ---- decoder (no bridge needed: Wl folded, no y feedback) ----
        if use_loops:
            with tc.For_i(0, t_steps * BC, ud * BC, hint_engines=(PE, DVE)) as iv:
                yt = ypool.tile([64, ud * BC], f32, tag="yt")
                for j in range(ud):
                    dec_step(yt, j)
                nc.sync.dma_start(y_ext[:, bass.ds(iv, ud * BC)], yt[:])
        else:
            for d0 in range(0, t_steps, ud):
                yt = ypool.tile([64, ud * BC], f32, tag="yt")
                for j in range(ud):
                    dec_step(yt, j)
                nc.sync.dma_start(y_ext[:, d0 * BC:(d0 + ud) * BC], yt[:])

        # ---- tail: y for the final hidden state -> slot T ----
        b_ay_t = psum.tile([128, 512], f32, tag="b_ay")
        gy_t = b_ay_t[0:64, 256:320]
        nc.tensor.matmul(gy_t, wtd[0][0:65, _Y0:_Y0 + 64],
                         rhs0d[0:65, :], start=True, stop=False)
        for k in range(1, 5):
            nc.tensor.matmul(gy_t, wtd[k][:, _Y0:_Y0 + 64],
                             hbf[:, (k - 1) * 64:k * 64], start=False, stop=(k == 4))
        ylast = ypool.tile([64, BC], f32, tag="ylast")
        nc.vector.tensor_copy(ylast[:], gy_t)
        nc.sync.dma_start(y_ext[:, t_steps * BC:(t_steps + 1) * BC], ylast[:])

    nc.compile()
    return nc


def run(nc, w_bf, x_cores, trace=False):
    """Execute on 8 cores; returns per-core y arrays and BassKernelResults."""
    from concourse.bass_utils import run_bass_kernel_spmd
    in_maps = [{"w": w_bf, "xh": x_cores[c]} for c in range(NCORES)]
    res = run_bass_kernel_spmd(nc, in_maps, core_ids=list(range(NCORES)),
                               trace=trace)
    return [res.results[c]["y"] for c in range(NCORES)], res


_NC_CACHE = {}


def kernel(x, Wi, Wh, bi, bh, Wl, bl, targets=None, target_seq_len=T,
           teacher_forcing_rate=0, **_unused):
    x = np.asarray(x, np.float32)
    assert x.shape == (B, S, I), x.shape
    assert int(target_seq_len) == T
    w_bf = _build_weights(np.asarray(Wi, np.float32), np.asarray(Wh, np.float32),
                          np.asarray(bi, np.float32), np.asarray(bh, np.float32),
                          np.asarray(Wl, np.float32), np.asarray(bl, np.float32))
    x_cores = [_build_x(x[c * BC:(c + 1) * BC]) for c in range(NCORES)]

    key = (S, T)
    if key not in _NC_CACHE:
        _NC_CACHE[key] = build_program(S, T)
    ys, _ = run(_NC_CACHE[key], w_bf, x_cores)

    out = np.empty((B, T, I), np.float32)
    for c in range(NCORES):
        yc = ys[c].reshape(64, T + 1, BC)[:, 1:, :]   # [I, T, BC]
        out[c * BC:(c + 1) * BC] = yc.transpose(2, 1, 0)
    return out


if __name__ == "__main__":
    import reference
    inputs = reference.setup_inputs()
    out = kernel(**{k: np.asarray(v) if hasattr(v, "shape") else v
                    for k, v in inputs.items()})
    print("kernel out", out.shape, out.dtype)


# revision 21
# speedup vs baseline: 1.1401x; 1.0038x over previous
"""Trainium2 Bass kernel for the GRU seq2seq AR model.

Model (reference): GRU encoder over S=1024 steps, then T=256 autoregressive
decoder steps (teacher_forcing_rate=0, so decoder input is always its own
previous output y = Wl @ h + bl).

Strategy:
  - Pure data parallel: batch 512 sharded 8 ways (64 rows/core), weights
    replicated, zero collectives.
  - Per step, fused matmuls compute gate pre-activations into four psum
    tiles gr|gz|gn|gh (each [128, 4x64batch]), K = [x(64); 1; h(512)]
    split in 5 k-tiles.  A ones-row folds all biases in.
  - Gate math uses the sigmoid_and_others ACT table (sigmoid+tanh, one
    table set):  r = sig(gr), nz = sig(-gz), u2 = r*gh, an = u2+gn (psum),
    tn = tanh(an), s = h-tn, p = nz*s, h' = h-p.  Split in H-halves for
    latency pipelining with the PE stream.
  - Decoder: Wl folded into gate weights (inp = Wl@h + bl always), so no
    serial y-feedback; y is output-only.
  - bf16 matmul inputs, fp32 PSUM, bf16 SBUF-side gate tensors.

Layouts (per core, BC = 64):
  w    DRAM [10, 128, 2112] bf16: k-tiles 0:5 encoder, 5:10 decoder.
  xh   DRAM [65, S*BC] bf16: rows 0:64 = x[t].T steps, row 64 = ones.
  y    DRAM [64, (T+1)*BC] f32: slot d holds Wl@h^{(d)}+bl ([I, BC] each).
"""

import numpy as np
import ml_dtypes

B, S, I, H, T = 512, 1024, 64, 512, 256
NCORES = 8
BC = B // NCORES
BF16 = ml_dtypes.bfloat16

# M-column blocks inside each 2112-col weight tile
_R0, _Z0, _N0, _H0, _Y0 = 0, 512, 1024, 1536, 2048
WCOLS = 2112


def _build_weights(Wi, Wh, bi, bh, Wl, bl):
    """lhsT tiles [10, 128, 2112] fp32 -> bf16.
    K rows: k0 = [x(64); ones(1)], k1..k4 = h chunks of 128.
    Tiles 0:5 = encoder, 5:10 = decoder (Wl folded).  No gate pre-scaling:
    r/z use real sigmoid."""
    w = np.zeros((10, 128, WCOLS), np.float32)

    def fill(base, Wx, bx_r, bx_z, bx_n, Whh, Win_, x_has_w):
        # z block pre-scaled by 0.5: the z nonlinearity rides in the tanh
        # op as tz = tanh(gz/2), z = 0.5 + 0.5*tz.
        if x_has_w:
            w[base, 0:64, _R0:_R0 + 512] = Wx.T[:, 0:512]
            w[base, 0:64, _Z0:_Z0 + 512] = 0.5 * Wx.T[:, 512:1024]
            w[base, 0:64, _N0:_N0 + 512] = Wx.T[:, 1024:1536]
        w[base, 64, _R0:_R0 + 512] = bx_r
        w[base, 64, _Z0:_Z0 + 512] = 0.5 * bx_z
        w[base, 64, _N0:_N0 + 512] = bx_n
        w[base, 64, _H0:_H0 + 512] = bh[1024:1536]
        w[base, 64, _Y0:_Y0 + 64] = bl
        for c in range(4):
            hs = slice(128 * c, 128 * (c + 1))
            w[base + 1 + c, :, _R0:_R0 + 512] = Whh.T[hs, 0:512]
            w[base + 1 + c, :, _Z0:_Z0 + 512] = 0.5 * Whh.T[hs, 512:1024]
            if Win_ is not None:
                w[base + 1 + c, :, _N0:_N0 + 512] = Win_.T[hs, :]
            w[base + 1 + c, :, _H0:_H0 + 512] = Wh[1024:1536].T[hs, :]
            w[base + 1 + c, :, _Y0:_Y0 + 64] = Wl.T[hs, :]

    # encoder: gi from x via k0; gh from h
    fill(0, Wi, (bi + bh)[0:512], (bi + bh)[512:1024], bi[1024:1536],
         Wh[0:1024], None, x_has_w=True)
    # decoder: inp = Wl@h + bl folded -> all through h rows
    Wc = Wh[0:1024] + Wi[0:1024] @ Wl
    Win = Wi[1024:1536] @ Wl
    fill(5, Wi,
         (bi + bh)[0:512] + Wi[0:512] @ bl,
         (bi + bh)[512:1024] + Wi[512:1024] @ bl,
         bi[1024:1536] + Wi[1024:1536] @ bl,
         Wc, Win, x_has_w=False)
    return w.astype(BF16)


def _build_x(xc):
    """xc [BC, S, I] -> [65, S*BC] bf16 with ones row."""
    s = xc.shape[1]
    xt = np.ones((65, s, BC), np.float32)
    xt[0:64] = xc.transpose(2, 1, 0)
    return np.ascontiguousarray(xt.reshape(65, s * BC)).astype(BF16)


def build_program(s_steps=S, t_steps=T, ue=32, ud=32, use_loops=True):
    """Build the Bass/Tile program (shared by all 8 cores)."""
    from contextlib import ExitStack
    import concourse.bass as bass
    import concourse.bacc as bacc
    import concourse.mybir as mybir
    import concourse.tile as tile

    f32 = mybir.dt.float32
    bf16 = mybir.dt.bfloat16
    TANH = mybir.ActivationFunctionType.Tanh
    SIG = mybir.ActivationFunctionType.Sigmoid
    MUL = mybir.AluOpType.mult
    ADD = mybir.AluOpType.add
    SUB = mybir.AluOpType.subtract

    assert s_steps % ue == 0 and t_steps % ud == 0

    nc = bacc.Bacc("TRN2", target_bir_lowering=False, debug=False,
                   num_devices=NCORES)
    w_ext = nc.declare_dram_parameter("w", [10, 128, WCOLS], bf16, isOutput=False)
    x_ext = nc.declare_dram_parameter("xh", [65, s_steps * BC], bf16, isOutput=False)
    y_ext = nc.declare_dram_parameter("y", [64, (t_steps + 1) * BC], f32, isOutput=True)

    with ExitStack() as ctx:
        tc = ctx.enter_context(tile.TileContext(nc))
        state = ctx.enter_context(tc.tile_pool(name="state", bufs=1))
        wpool = ctx.enter_context(tc.tile_pool(name="wpool", bufs=1))
        xpool = ctx.enter_context(tc.tile_pool(name="xpool", bufs=2))
        ypool = ctx.enter_context(tc.tile_pool(name="ypool", bufs=2))
        gp = ctx.enter_context(tc.tile_pool(name="gates", bufs=2))
        psum = ctx.enter_context(tc.tile_pool(name="psum", bufs=2, space="PSUM"))

        wte, wtd = [], []
        for k in range(10):
            t_ = wpool.tile([128, WCOLS], bf16, tag=f"w{k}")
            nc.sync.dma_start(t_[:], w_ext[k, :, :])
            (wte if k < 5 else wtd).append(t_)

        hbf = state.tile([128, 256], bf16, tag="hbf")    # h.T chunks (bf16)
        rhs0d = state.tile([65, BC], bf16, tag="rhs0d")  # decoder k0 = [0...; 1]
        nc.vector.memset(hbf[:], 0.0)
        nc.vector.memset(rhs0d[:], 0.0)
        nc.vector.memset(rhs0d[64:65, :], 1.0)

        HA, HB = slice(0, 128), slice(128, 256)

        def emit_mms(b_r, b_hn, b_az, b_y, rhs0, wt, enc, want_y):
            """Gate matmuls.  Banks by chain deadline: b_r holds r (group
            closes right after the k3/k4 r MMs so the sigmoid starts
            early), b_hn holds h_n|n, b_az holds the DVE-written an region
            (cols 0:256) plus z' (cols 256:512, 0.5-scaled), b_y (dec) y.
            start=True only on the first MM per bank (clears the whole
            bank); stop=True only on the last MM per bank.
            Emission order: k0 prefetch | r,h on k1k2 | r,h on k3k4
            (chain-critical, right at h'B arrival) | z (late deadline)."""
            hk = lambda k: hbf[:, (k - 1) * 64:k * 64]
            k0 = wt[0][0:65, :]

            def mm4(ps, pcol0, wcol0, k, start, stop):
                # start applies only to the first (m=0) MM, stop to the last
                for m in range(4):
                    lhs = (k0 if k == 0 else wt[k])
                    rr = (rhs0 if k == 0 else hk(k))
                    nc.tensor.matmul(ps[:, pcol0 + 64 * m:pcol0 + 64 * m + 64],
                                     lhs[:, wcol0 + 128 * m:wcol0 + 128 * m + 128],
                                     rr, start=(start and m == 0),
                                     stop=(stop and m == 3))

            # k0 (x rows for encoder; bias row always); first MM per bank
            mm4(b_r, 0, _R0, 0, True, False)
            mm4(b_hn, 0, _H0, 0, True, False)
            mm4(b_az, 256, _Z0, 0, True, False)
            mm4(b_hn, 256, _N0, 0, False, False)
            if want_y:
                nc.tensor.matmul(b_y[:, :], k0[:, _Y0:_Y0 + 64],
                                 rhs0, start=True, stop=False)
            # k1/k2 groups (hbf half-A readers): r, h_n (+n dec)
            for k in (1, 2):
                mm4(b_r, 0, _R0, k, False, False)
            for k in (1, 2):
                mm4(b_hn, 0, _H0, k, False, False)
            if not enc:
                for k in (1, 2):
                    mm4(b_hn, 256, _N0, k, False, False)
            # k3/k4 groups (hbf half-B readers): r first = chain head
            mm4(b_r, 0, _R0, 3, False, False)
            mm4(b_r, 0, _R0, 4, False, True)
            mm4(b_hn, 0, _H0, 3, False, False)
            mm4(b_hn, 0, _H0, 4, False, enc)
            if not enc:
                mm4(b_hn, 256, _N0, 3, False, False)
                mm4(b_hn, 256, _N0, 4, False, True)
            # z: late deadline (consumed by the taz tanh mid-chain)
            for k in (1, 2):
                mm4(b_az, 256, _Z0, k, False, False)
            mm4(b_az, 256, _Z0, 3, False, False)
            mm4(b_az, 256, _Z0, 4, False, True)
            if want_y:
                for k in (1, 2, 3, 4):
                    nc.tensor.matmul(b_y[:, :], wt[k][:, _Y0:_Y0 + 64],
                                     hk(k), start=False, stop=(k == 4))

        def emit_gates(b_r, b_hn, b_az, b_y, ytile=None, yslot=0):
            """Gate math in H-halves:
              r = sig(gr);  u2 = r*gh;  an = u2+gn  (-> psum, next to z');
              [tn|tz] = tanh([an|gz/2]) in ONE strided ACT per half;
              nz2 = 0.5-0.5*tz (=1-z);  s = tn-h;  p = nz2*s;  h' = h+p."""
            gr = b_r[:, 0:256]
            gh = b_hn[:, 0:256]
            gn = b_hn[:, 256:512]
            an = b_az[:, 0:256]
            r = gp.tile([128, 256], bf16, tag="r")
            u2 = gp.tile([128, 256], bf16, tag="u2")
            taz = gp.tile([128, 512], bf16, tag="taz")  # [tnA|tzA|tnB|tzB]
            nz2 = gp.tile([128, 256], bf16, tag="nz2")
            s = gp.tile([128, 256], bf16, tag="s")
            p = gp.tile([128, 256], bf16, tag="p")

            # ACT: full-width sigmoid for r
            nc.scalar.activation(r[:], gr, SIG)
            # DVE head: u2/an per half (an lands in psum next to z')
            nc.vector.tensor_tensor(u2[:, HA], r[:, HA], gh[:, HA], MUL)
            nc.vector.tensor_tensor(an[:, HA], u2[:, HA], gn[:, HA], ADD)
            nc.vector.tensor_tensor(u2[:, HB], r[:, HB], gh[:, HB], MUL)
            nc.vector.tensor_tensor(an[:, HB], u2[:, HB], gn[:, HB], ADD)
            # ONE tanh per half covering [an_half | z'_half]: strided view
            # of b_az [128,512] as [128, (an|z'), (A|B), 128]
            bz4 = b_az[:, 0:512].rearrange("p (a b c) -> p a b c", a=2, b=2, c=128)
            to2 = lambda t: t.rearrange("p (a c) -> p a c", a=2, c=128)
            nc.scalar.activation(to2(taz[:, 0:256]), bz4[:, :, 0, :], TANH)
            nc.scalar.activation(to2(taz[:, 256:512]), bz4[:, :, 1, :], TANH)
            # DVE tail: h' = h + nz2*(tn - h)
            tnA, tzA = taz[:, 0:128], taz[:, 128:256]
            tnB, tzB = taz[:, 256:384], taz[:, 384:512]
            nc.vector.tensor_scalar(nz2[:, HA], tzA, -0.5, 0.5, MUL, ADD)
            nc.vector.tensor_tensor(s[:, HA], tnA, hbf[:, HA], SUB)
            nc.vector.tensor_tensor(p[:, HA], nz2[:, HA], s[:, HA], MUL)
            nc.vector.tensor_tensor(hbf[:, HA], hbf[:, HA], p[:, HA], ADD)
            nc.vector.tensor_scalar(nz2[:, HB], tzB, -0.5, 0.5, MUL, ADD)
            nc.vector.tensor_tensor(s[:, HB], tnB, hbf[:, HB], SUB)
            nc.vector.tensor_tensor(p[:, HB], nz2[:, HB], s[:, HB], MUL)
            nc.vector.tensor_tensor(hbf[:, HB], hbf[:, HB], p[:, HB], ADD)
            if ytile is not None:
                nc.vector.tensor_copy(
                    ytile[:, yslot * BC:(yslot + 1) * BC], b_y[:, :])

        def enc_step(rhs0):
            b_r = psum.tile([128, 256], f32, tag="b_r")
            b_hn = psum.tile([128, 512], f32, tag="b_hn")
            b_az = psum.tile([128, 512], f32, tag="b_az")
            emit_mms(b_r, b_hn, b_az, None, rhs0, wte, enc=True, want_y=False)
            emit_gates(b_r, b_hn, b_az, None)

        def dec_step(ytile, yslot):
            b_r = psum.tile([128, 256], f32, tag="b_r")
            b_hn = psum.tile([128, 512], f32, tag="b_hn")
            b_az = psum.tile([128, 512], f32, tag="b_az")
            b_y = psum.tile([64, 64], f32, tag="b_y")
            emit_mms(b_r, b_hn, b_az, b_y, rhs0d[0:65, :], wtd, enc=False, want_y=True)
            emit_gates(b_r, b_hn, b_az, b_y, ytile=ytile, yslot=yslot)

        PE = mybir.EngineType.PE
        DVE = mybir.EngineType.DVE

        # ---- encoder ----
        if use_loops:
            with tc.For_i(0, s_steps * BC, ue * BC, hint_engines=(PE, DVE)) as iv:
                xch = xpool.tile([65, ue * BC], bf16, tag="xch")
                nc.sync.dma_start(xch[:], x_ext[:, bass.ds(iv, ue * BC)])
                for j in range(ue):
                    enc_step(xch[:, j * BC:(j + 1) * BC])
        else:
            for i0 in range(0, s_steps, ue):
                xch = xpool.tile([65, ue * BC], bf16, tag="xch")
                nc.sync.dma_start(xch[:], x_ext[:, i0 * BC:(i0 + ue) * BC])
                for j in range(ue):
                    enc_step(xch[:, j * BC:(j + 1) * BC])

        # ---- decoder (no bridge needed: Wl folded, no y feedback) ----
        if use_loops:
            with tc.For_i(0, t_steps * BC, ud * BC, hint_engines=(PE, DVE)) as iv:
                yt = ypool.tile([64, ud * BC], f32, tag="yt")
                for j in range(ud):
                    dec_step(yt, j)
                nc.sync.dma_start(y_ext[:, bass.ds(iv, ud * BC)], yt[:])
        else:
            for d0 in range(0, t_steps, ud):
                yt = ypool.tile([64, ud * BC], f32, tag="yt")
                for j in range(ud):
                    dec_step(yt, j)
                nc.sync.dma_start(y_ext[:, d0 * BC:(d0 + ud) * BC], yt[:])

        # ---- tail: y for the final hidden state -> slot T ----
        b_y_t = psum.tile([64, 64], f32, tag="b_y")
        gy_t = b_y_t[:, :]
        nc.tensor.matmul(gy_t, wtd[0][0:65, _Y0:_Y0 + 64],
                         rhs0d[0:65, :], start=True, stop=False)
        for k in range(1, 5):
            nc.tensor.matmul(gy_t, wtd[k][:, _Y0:_Y0 + 64],
                             hbf[:, (k - 1) * 64:k * 64], start=False, stop=(k == 4))
        ylast = ypool.tile([64, BC], f32, tag="ylast")
        nc.vector.tensor_copy(ylast[:], gy_t)
        nc.sync.dma_start(y_ext[:, t_steps * BC:(t_steps + 1) * BC], ylast[:])

    nc.compile()
    return nc


def run(nc, w_bf, x_cores, trace=False):
    """Execute on 8 cores; returns per-core y arrays and BassKernelResults."""
    from concourse.bass_utils import run_bass_kernel_spmd
    in_maps = [{"w": w_bf, "xh": x_cores[c]} for c in range(NCORES)]
    res = run_bass_kernel_spmd(nc, in_maps, core_ids=list(range(NCORES)),
                               trace=trace)
    return [res.results[c]["y"] for c in range(NCORES)], res


_NC_CACHE = {}


def kernel(x, Wi, Wh, bi, bh, Wl, bl, targets=None, target_seq_len=T,
           teacher_forcing_rate=0, **_unused):
    x = np.asarray(x, np.float32)
    assert x.shape == (B, S, I), x.shape
    assert int(target_seq_len) == T
    w_bf = _build_weights(np.asarray(Wi, np.float32), np.asarray(Wh, np.float32),
                          np.asarray(bi, np.float32), np.asarray(bh, np.float32),
                          np.asarray(Wl, np.float32), np.asarray(bl, np.float32))
    x_cores = [_build_x(x[c * BC:(c + 1) * BC]) for c in range(NCORES)]

    key = (S, T)
    if key not in _NC_CACHE:
        _NC_CACHE[key] = build_program(S, T)
    ys, _ = run(_NC_CACHE[key], w_bf, x_cores)

    out = np.empty((B, T, I), np.float32)
    for c in range(NCORES):
        yc = ys[c].reshape(64, T + 1, BC)[:, 1:, :]   # [I, T, BC]
        out[c * BC:(c + 1) * BC] = yc.transpose(2, 1, 0)
    return out


if __name__ == "__main__":
    import reference
    inputs = reference.setup_inputs()
    out = kernel(**{k: np.asarray(v) if hasattr(v, "shape") else v
                    for k, v in inputs.items()})
    print("kernel out", out.shape, out.dtype)


# revision 23
# speedup vs baseline: 1.1532x; 1.0115x over previous
"""Trainium2 Bass kernel for the GRU seq2seq AR model.

Model (reference): GRU encoder over S=1024 steps, then T=256 autoregressive
decoder steps (teacher_forcing_rate=0, so decoder input is always its own
previous output y = Wl @ h + bl).

Strategy:
  - Pure data parallel: batch 512 sharded 8 ways (64 rows/core), weights
    replicated, zero collectives.
  - Per step, fused matmuls compute gate pre-activations into four psum
    tiles gr|gz|gn|gh (each [128, 4x64batch]), K = [x(64); 1; h(512)]
    split in 5 k-tiles.  A ones-row folds all biases in.
  - Gate math uses the sigmoid_and_others ACT table (sigmoid+tanh, one
    table set):  r = sig(gr), nz = sig(-gz), u2 = r*gh, an = u2+gn (psum),
    tn = tanh(an), s = h-tn, p = nz*s, h' = h-p.  Split in H-halves for
    latency pipelining with the PE stream.
  - Decoder: Wl folded into gate weights (inp = Wl@h + bl always), so no
    serial y-feedback; y is output-only.
  - bf16 matmul inputs, fp32 PSUM, bf16 SBUF-side gate tensors.

Layouts (per core, BC = 64):
  w    DRAM [10, 128, 2112] bf16: k-tiles 0:5 encoder, 5:10 decoder.
  xh   DRAM [65, S*BC] bf16: rows 0:64 = x[t].T steps, row 64 = ones.
  y    DRAM [64, (T+1)*BC] f32: slot d holds Wl@h^{(d)}+bl ([I, BC] each).
"""

import numpy as np
import ml_dtypes

B, S, I, H, T = 512, 1024, 64, 512, 256
NCORES = 8
BC = B // NCORES
BF16 = ml_dtypes.bfloat16

# M-column blocks inside each 2112-col weight tile
_R0, _Z0, _N0, _H0, _Y0 = 0, 512, 1024, 1536, 2048
WCOLS = 2112


def _build_weights(Wi, Wh, bi, bh, Wl, bl):
    """lhsT tiles [10, 128, 2112] fp32 -> bf16.
    K rows: k0 = [x(64); ones(1)], k1..k4 = h chunks of 128.
    Tiles 0:5 = encoder, 5:10 = decoder (Wl folded).  No gate pre-scaling:
    r/z use real sigmoid."""
    w = np.zeros((10, 128, WCOLS), np.float32)

    def fill(base, Wx, bx_r, bx_z, bx_n, Whh, Win_, x_has_w):
        # z block pre-scaled by 0.5: the z nonlinearity rides in the tanh
        # op as tz = tanh(gz/2), z = 0.5 + 0.5*tz.
        if x_has_w:
            w[base, 0:64, _R0:_R0 + 512] = Wx.T[:, 0:512]
            w[base, 0:64, _Z0:_Z0 + 512] = 0.5 * Wx.T[:, 512:1024]
            w[base, 0:64, _N0:_N0 + 512] = Wx.T[:, 1024:1536]
        w[base, 64, _R0:_R0 + 512] = bx_r
        w[base, 64, _Z0:_Z0 + 512] = 0.5 * bx_z
        w[base, 64, _N0:_N0 + 512] = bx_n
        w[base, 64, _H0:_H0 + 512] = bh[1024:1536]
        w[base, 64, _Y0:_Y0 + 64] = bl
        for c in range(4):
            hs = slice(128 * c, 128 * (c + 1))
            w[base + 1 + c, :, _R0:_R0 + 512] = Whh.T[hs, 0:512]
            w[base + 1 + c, :, _Z0:_Z0 + 512] = 0.5 * Whh.T[hs, 512:1024]
            if Win_ is not None:
                w[base + 1 + c, :, _N0:_N0 + 512] = Win_.T[hs, :]
            w[base + 1 + c, :, _H0:_H0 + 512] = Wh[1024:1536].T[hs, :]
            w[base + 1 + c, :, _Y0:_Y0 + 64] = Wl.T[hs, :]

    # encoder: gi from x via k0; gh from h
    fill(0, Wi, (bi + bh)[0:512], (bi + bh)[512:1024], bi[1024:1536],
         Wh[0:1024], None, x_has_w=True)
    # decoder: inp = Wl@h + bl folded -> all through h rows
    Wc = Wh[0:1024] + Wi[0:1024] @ Wl
    Win = Wi[1024:1536] @ Wl
    fill(5, Wi,
         (bi + bh)[0:512] + Wi[0:512] @ bl,
         (bi + bh)[512:1024] + Wi[512:1024] @ bl,
         bi[1024:1536] + Wi[1024:1536] @ bl,
         Wc, Win, x_has_w=False)
    return w.astype(BF16)


def _build_x(xc):
    """xc [BC, S, I] -> [65, S*BC] bf16 with ones row."""
    s = xc.shape[1]
    xt = np.ones((65, s, BC), np.float32)
    xt[0:64] = xc.transpose(2, 1, 0)
    return np.ascontiguousarray(xt.reshape(65, s * BC)).astype(BF16)


def build_program(s_steps=S, t_steps=T, ue=64, ud=64, use_loops=True):
    """Build the Bass/Tile program (shared by all 8 cores)."""
    from contextlib import ExitStack
    import concourse.bass as bass
    import concourse.bacc as bacc
    import concourse.mybir as mybir
    import concourse.tile as tile

    f32 = mybir.dt.float32
    bf16 = mybir.dt.bfloat16
    TANH = mybir.ActivationFunctionType.Tanh
    SIG = mybir.ActivationFunctionType.Sigmoid
    MUL = mybir.AluOpType.mult
    ADD = mybir.AluOpType.add
    SUB = mybir.AluOpType.subtract

    assert s_steps % ue == 0 and t_steps % ud == 0

    nc = bacc.Bacc("TRN2", target_bir_lowering=False, debug=False,
                   num_devices=NCORES)
    w_ext = nc.declare_dram_parameter("w", [10, 128, WCOLS], bf16, isOutput=False)
    x_ext = nc.declare_dram_parameter("xh", [65, s_steps * BC], bf16, isOutput=False)
    y_ext = nc.declare_dram_parameter("y", [64, (t_steps + 1) * BC], f32, isOutput=True)

    with ExitStack() as ctx:
        tc = ctx.enter_context(tile.TileContext(nc))
        state = ctx.enter_context(tc.tile_pool(name="state", bufs=1))
        wpool = ctx.enter_context(tc.tile_pool(name="wpool", bufs=1))
        xpool = ctx.enter_context(tc.tile_pool(name="xpool", bufs=2))
        ypool = ctx.enter_context(tc.tile_pool(name="ypool", bufs=2))
        gp = ctx.enter_context(tc.tile_pool(name="gates", bufs=2))
        psum = ctx.enter_context(tc.tile_pool(name="psum", bufs=2, space="PSUM"))

        wte, wtd = [], []
        for k in range(10):
            t_ = wpool.tile([128, WCOLS], bf16, tag=f"w{k}")
            nc.sync.dma_start(t_[:], w_ext[k, :, :])
            (wte if k < 5 else wtd).append(t_)

        hbf = state.tile([128, 256], bf16, tag="hbf")    # h.T chunks (bf16)
        rhs0d = state.tile([65, BC], bf16, tag="rhs0d")  # decoder k0 = [0...; 1]
        nc.vector.memset(hbf[:], 0.0)
        nc.vector.memset(rhs0d[:], 0.0)
        nc.vector.memset(rhs0d[64:65, :], 1.0)

        HA, HB = slice(0, 128), slice(128, 256)

        def emit_mms(b_r, b_hn, b_az, b_y, rhs0, wt, enc, want_y):
            """Gate matmuls.  Banks by chain deadline: b_r holds r (group
            closes right after the k3/k4 r MMs so the sigmoid starts
            early), b_hn holds h_n|n, b_az holds the DVE-written an region
            (cols 0:256) plus z' (cols 256:512, 0.5-scaled), b_y (dec) y.
            start=True only on the first MM per bank (clears the whole
            bank); stop=True only on the last MM per bank.
            Emission order: k0 prefetch | r,h on k1k2 | r,h on k3k4
            (chain-critical, right at h'B arrival) | z (late deadline)."""
            hk = lambda k: hbf[:, (k - 1) * 64:k * 64]
            k0 = wt[0][0:65, :]

            def mm4(ps, pcol0, wcol0, k, start, stop):
                # start applies only to the first (m=0) MM, stop to the last
                for m in range(4):
                    lhs = (k0 if k == 0 else wt[k])
                    rr = (rhs0 if k == 0 else hk(k))
                    nc.tensor.matmul(ps[:, pcol0 + 64 * m:pcol0 + 64 * m + 64],
                                     lhs[:, wcol0 + 128 * m:wcol0 + 128 * m + 128],
                                     rr, start=(start and m == 0),
                                     stop=(stop and m == 3))

            # k0 (x rows for encoder; bias row always); first MM per bank.
            # High priority: k0 has no h dependency — the scheduler should
            # place it during the previous step's gate chain.
            with tc.high_priority(offset=150):
                mm4(b_r, 0, _R0, 0, True, False)
                mm4(b_hn, 0, _H0, 0, True, False)
                mm4(b_az, 256, _Z0, 0, True, False)
                mm4(b_hn, 256, _N0, 0, False, False)
                if want_y:
                    nc.tensor.matmul(b_y[:, :], k0[:, _Y0:_Y0 + 64],
                                     rhs0, start=True, stop=False)
            # k1/k2 groups (hbf half-A readers): r, h_n (+n dec)
            for k in (1, 2):
                mm4(b_r, 0, _R0, k, False, False)
            for k in (1, 2):
                mm4(b_hn, 0, _H0, k, False, False)
            if not enc:
                for k in (1, 2):
                    mm4(b_hn, 256, _N0, k, False, False)
            # k3/k4 groups (hbf half-B readers): r first = chain head
            mm4(b_r, 0, _R0, 3, False, False)
            mm4(b_r, 0, _R0, 4, False, True)
            mm4(b_hn, 0, _H0, 3, False, False)
            mm4(b_hn, 0, _H0, 4, False, enc)
            if not enc:
                mm4(b_hn, 256, _N0, 3, False, False)
                mm4(b_hn, 256, _N0, 4, False, True)
            # z: late deadline (consumed by the taz tanh mid-chain)
            for k in (1, 2):
                mm4(b_az, 256, _Z0, k, False, False)
            mm4(b_az, 256, _Z0, 3, False, False)
            mm4(b_az, 256, _Z0, 4, False, True)
            if want_y:
                for k in (1, 2, 3, 4):
                    nc.tensor.matmul(b_y[:, :], wt[k][:, _Y0:_Y0 + 64],
                                     hk(k), start=False, stop=(k == 4))

        def emit_gates(b_r, b_hn, b_az, b_y, ytile=None, yslot=0):
            """Gate math in H-halves:
              r = sig(gr);  u2 = r*gh;  an = u2+gn  (-> psum, next to z');
              [tn|tz] = tanh([an|gz/2]) in ONE strided ACT per half;
              nz2 = 0.5-0.5*tz (=1-z);  s = tn-h;  p = nz2*s;  h' = h+p."""
            gr = b_r[:, 0:256]
            gh = b_hn[:, 0:256]
            gn = b_hn[:, 256:512]
            an = b_az[:, 0:256]
            r = gp.tile([128, 256], bf16, tag="r")
            u2 = gp.tile([128, 256], bf16, tag="u2")
            taz = gp.tile([128, 512], bf16, tag="taz")  # [tnA|tzA|tnB|tzB]
            nz2 = gp.tile([128, 256], bf16, tag="nz2")
            s = gp.tile([128, 256], bf16, tag="s")
            p = gp.tile([128, 256], bf16, tag="p")

            # ACT: full-width sigmoid for r
            nc.scalar.activation(r[:], gr, SIG)
            # DVE head: u2/an per half (an lands in psum next to z')
            nc.vector.tensor_tensor(u2[:, HA], r[:, HA], gh[:, HA], MUL)
            nc.vector.tensor_tensor(an[:, HA], u2[:, HA], gn[:, HA], ADD)
            nc.vector.tensor_tensor(u2[:, HB], r[:, HB], gh[:, HB], MUL)
            nc.vector.tensor_tensor(an[:, HB], u2[:, HB], gn[:, HB], ADD)
            # ONE tanh per half covering [an_half | z'_half]: strided view
            # of b_az [128,512] as [128, (an|z'), (A|B), 128]
            bz4 = b_az[:, 0:512].rearrange("p (a b c) -> p a b c", a=2, b=2, c=128)
            to2 = lambda t: t.rearrange("p (a c) -> p a c", a=2, c=128)
            nc.scalar.activation(to2(taz[:, 0:256]), bz4[:, :, 0, :], TANH)
            nc.scalar.activation(to2(taz[:, 256:512]), bz4[:, :, 1, :], TANH)
            # DVE tail: h' = h + nz2*(tn - h)
            tnA, tzA = taz[:, 0:128], taz[:, 128:256]
            tnB, tzB = taz[:, 256:384], taz[:, 384:512]
            nc.vector.tensor_scalar(nz2[:, HA], tzA, -0.5, 0.5, MUL, ADD)
            nc.vector.tensor_tensor(s[:, HA], tnA, hbf[:, HA], SUB)
            nc.vector.tensor_tensor(p[:, HA], nz2[:, HA], s[:, HA], MUL)
            nc.vector.tensor_tensor(hbf[:, HA], hbf[:, HA], p[:, HA], ADD)
            nc.vector.tensor_scalar(nz2[:, HB], tzB, -0.5, 0.5, MUL, ADD)
            nc.vector.tensor_tensor(s[:, HB], tnB, hbf[:, HB], SUB)
            nc.vector.tensor_tensor(p[:, HB], nz2[:, HB], s[:, HB], MUL)
            nc.vector.tensor_tensor(hbf[:, HB], hbf[:, HB], p[:, HB], ADD)
            if ytile is not None:
                nc.vector.tensor_copy(
                    ytile[:, yslot * BC:(yslot + 1) * BC], b_y[:, :])

        def enc_step(rhs0):
            b_r = psum.tile([128, 256], f32, tag="b_r")
            b_hn = psum.tile([128, 512], f32, tag="b_hn")
            b_az = psum.tile([128, 512], f32, tag="b_az")
            emit_mms(b_r, b_hn, b_az, None, rhs0, wte, enc=True, want_y=False)
            emit_gates(b_r, b_hn, b_az, None)

        def dec_step(ytile, yslot):
            b_r = psum.tile([128, 256], f32, tag="b_r")
            b_hn = psum.tile([128, 512], f32, tag="b_hn")
            b_az = psum.tile([128, 512], f32, tag="b_az")
            b_y = psum.tile([64, 64], f32, tag="b_y")
            emit_mms(b_r, b_hn, b_az, b_y, rhs0d[0:65, :], wtd, enc=False, want_y=True)
            emit_gates(b_r, b_hn, b_az, b_y, ytile=ytile, yslot=yslot)

        PE = mybir.EngineType.PE
        DVE = mybir.EngineType.DVE

        # ---- encoder ----
        if use_loops:
            with tc.For_i(0, s_steps * BC, ue * BC, hint_engines=(PE, DVE)) as iv:
                xch = xpool.tile([65, ue * BC], bf16, tag="xch")
                nc.sync.dma_start(xch[:], x_ext[:, bass.ds(iv, ue * BC)])
                for j in range(ue):
                    enc_step(xch[:, j * BC:(j + 1) * BC])
        else:
            for i0 in range(0, s_steps, ue):
                xch = xpool.tile([65, ue * BC], bf16, tag="xch")
                nc.sync.dma_start(xch[:], x_ext[:, i0 * BC:(i0 + ue) * BC])
                for j in range(ue):
                    enc_step(xch[:, j * BC:(j + 1) * BC])

        # ---- decoder (no bridge needed: Wl folded, no y feedback) ----
        if use_loops:
            with tc.For_i(0, t_steps * BC, ud * BC, hint_engines=(PE, DVE)) as iv:
                yt = ypool.tile([64, ud * BC], f32, tag="yt")
                for j in range(ud):
                    dec_step(yt, j)
                nc.sync.dma_start(y_ext[:, bass.ds(iv, ud * BC)], yt[:])
        else:
            for d0 in range(0, t_steps, ud):
                yt = ypool.tile([64, ud * BC], f32, tag="yt")
                for j in range(ud):
                    dec_step(yt, j)
                nc.sync.dma_start(y_ext[:, d0 * BC:(d0 + ud) * BC], yt[:])

        # ---- tail: y for the final hidden state -> slot T ----
        b_y_t = psum.tile([64, 64], f32, tag="b_y")
        gy_t = b_y_t[:, :]
        nc.tensor.matmul(gy_t, wtd[0][0:65, _Y0:_Y0 + 64],
                         rhs0d[0:65, :], start=True, stop=False)
        for k in range(1, 5):
            nc.tensor.matmul(gy_t, wtd[k][:, _Y0:_Y0 + 64],
                             hbf[:, (k - 1) * 64:k * 64], start=False, stop=(k == 4))
        ylast = ypool.tile([64, BC], f32, tag="ylast")
        nc.vector.tensor_copy(ylast[:], gy_t)
        nc.sync.dma_start(y_ext[:, t_steps * BC:(t_steps + 1) * BC], ylast[:])

    nc.compile()
    return nc


def run(nc, w_bf, x_cores, trace=False):
    """Execute on 8 cores; returns per-core y arrays and BassKernelResults."""
    from concourse.bass_utils import run_bass_kernel_spmd
    in_maps = [{"w": w_bf, "xh": x_cores[c]} for c in range(NCORES)]
    res = run_bass_kernel_spmd(nc, in_maps, core_ids=list(range(NCORES)),
                               trace=trace)
    return [res.results[c]["y"] for c in range(NCORES)], res


_NC_CACHE = {}


def kernel(x, Wi, Wh, bi, bh, Wl, bl, targets=None, target_seq_len=T,
           teacher_forcing_rate=0, **_unused):
    x = np.asarray(x, np.float32)
    assert x.shape == (B, S, I), x.shape
    assert int(target_seq_len) == T
    w_bf = _build_weights(np.asarray(Wi, np.float32), np.asarray(Wh, np.float32),
                          np.asarray(bi, np.float32), np.asarray(bh, np.float32),
                          np.asarray(Wl, np.float32), np.asarray(bl, np.float32))
    x_cores = [_build_x(x[c * BC:(c + 1) * BC]) for c in range(NCORES)]

    key = (S, T)
    if key not in _NC_CACHE:
        _NC_CACHE[key] = build_program(S, T)
    ys, _ = run(_NC_CACHE[key], w_bf, x_cores)

    out = np.empty((B, T, I), np.float32)
    for c in range(NCORES):
        yc = ys[c].reshape(64, T + 1, BC)[:, 1:, :]   # [I, T, BC]
        out[c * BC:(c + 1) * BC] = yc.transpose(2, 1, 0)
    return out


if __name__ == "__main__":
    import reference
    inputs = reference.setup_inputs()
    out = kernel(**{k: np.asarray(v) if hasattr(v, "shape") else v
                    for k, v in inputs.items()})
    print("kernel out", out.shape, out.dtype)
